# revision 1
# baseline (speedup 1.0000x reference)
"""Trainium2 Bass kernel for nn_AttenConv (sparse_attention).

Math (see reference):
  e_j = user_emb @ W ; e_k = item_emb @ W
  u_neigh = segsum_row(adj, e_k) ; i_neigh = segsum_col(adj, e_j)
  out = softmax(u_neigh @ i_neigh.T, axis=1) @ e_k @ W

Kernel formulation:
  A (dense [U,I], fp16) built on host from COO.  W commutes with segsum:
    u_neigh = (A @ item_emb) @ W ; i_neigh = (A.T @ user_emb) @ W
  ekW = item_emb @ (W @ W).
  Sharding: users row-sharded (u-side), items col-sharded (i-side segsum)
  with AllGather of i_neighT; attention fully user-sharded.

v3 schedule (vs v1):
  - a_i streamed first at full HBM bw; AllGather (fp16 payload) fires
    right after the i-side segsum and overlaps the u-side segsum.
  - single-pass fp16 logits (no hi/lo): fp32r measured 2.7-3.4 cyc/row
    on real HW, so everything stays fp16.
  - pass1 row-max: vector reduce_max directly off the logits PSUM; no
    scalar copies, no max tree; gathered i_neighT DMAs straight into
    the big fp16 tile.
  - -max bias still injected as a K=1 ones x negm matmul (cheapest
    per-column broadcast on this machine).
"""
import sys
sys.path.insert(0, '/opt/trn_rl_repo')
import numpy as np

import concourse.bass as bass
import concourse.mybir as mybir
import concourse.tile as tile
from concourse import bacc, bass_utils
from concourse.masks import make_identity

F32 = mybir.dt.float32
F32R = mybir.dt.float32r
F16 = mybir.dt.float16
AX = mybir.AxisListType.X
AF = mybir.ActivationFunctionType

D = 128
P = 128


def build_nc(nu, ni, cores):
    UL = nu // cores          # users per core
    IL = ni // cores          # items per core (i-side segsum shard)
    UHW = min(512, UL)        # user window for pass1/2
    NUH = UL // UHW           # user windows per core
    UBW = UHW // P            # 128-user blocks per window
    NIC = ni // P             # 128-item chunks
    NUC = nu // P             # 128-user chunks (i-side segsum contraction)
    ILW = min(512, IL)
    NIW = IL // ILW
    NIB = ni // 512           # 512-item blocks (pass-1 rhs)

    nc = bacc.Bacc("TRN2", debug=False, num_devices=cores)

    a_ut = nc.dram_tensor("a_ut", [ni, UL], F16, kind="ExternalInput")
    a_i = nc.dram_tensor("a_i", [nu, IL], F16, kind="ExternalInput")
    item16 = nc.dram_tensor("item16", [P, ni], F16, kind="ExternalInput")
    itemT16 = nc.dram_tensor("itemT16", [D, ni], F16, kind="ExternalInput")
    user16 = nc.dram_tensor("user16", [P, nu], F16, kind="ExternalInput")
    w_in = nc.dram_tensor("w", [D, D], F32, kind="ExternalInput")
    wt_in = nc.dram_tensor("wt", [D, D], F32, kind="ExternalInput")
    out_sl = nc.dram_tensor("out_sl", [UL, D], F32, kind="ExternalOutput")

    cc_in = nc.dram_tensor("cc_in", [D, IL], F16)
    cc_out = nc.dram_tensor("cc_out", [cores * D, IL], F16,
                            addr_space="Shared")

    with tile.TileContext(nc) as tc:
        with (
            tc.tile_pool(name="const", bufs=1) as cpool,
            tc.tile_pool(name="big", bufs=1) as bigp,
            tc.tile_pool(name="astream", bufs=5) as apool,
            tc.tile_pool(name="work", bufs=1) as wpool,
            tc.tile_pool(name="negmp", bufs=2) as negmp,
            tc.tile_pool(name="mt", bufs=2) as mtpool,
            tc.tile_pool(name="at", bufs=4) as atpool,
            tc.tile_pool(name="ps1", bufs=1, space="PSUM") as ps1pool,
            tc.tile_pool(name="ps_seg", bufs=1, space="PSUM") as ps_seg,
            tc.tile_pool(name="ps_out", bufs=1, space="PSUM") as ps_out,
        ):
            # ---------- constants + user emb (needed by i-side segsum) ----
            w_sb = cpool.tile([D, D], F32)
            wt_sb = cpool.tile([D, D], F32)
            nc.sync.dma_start(out=w_sb[:], in_=w_in[:])
            nc.sync.dma_start(out=wt_sb[:], in_=wt_in[:])
            ident = cpool.tile([P, P], F32)
            make_identity(nc, ident[:])
            ident16 = cpool.tile([P, P], F16)
            make_identity(nc, ident16[:])
            ones1 = cpool.tile([1, P], F16)
            nc.gpsimd.memset(ones1[:], 1.0)
            user_sb = bigp.tile([P, nu], F16)
            nc.sync.dma_start(out=user_sb[:], in_=user16[:])

            # ---------- i-side segsum (feeds AllGather; a_i has DMA
            # priority: no other bulk DMA is issued before it) ----------
            GRP = 4 if NUC % 4 == 0 else 1
            ps_iw = [ps_seg.tile([D, ILW], F32, tag=f"segps{iw}",
                                 name=f"psiw{iw}")
                     for iw in range(NIW)]
            item_sb = bigp.tile([P, ni], F16)
            itemT_sb = bigp.tile([P, ni], F16)
            for ug in range(NUC // GRP):
                if ug == NUC // GRP - 6:
                    # issue late in the a_i stream: ready right when the
                    # u-side segsum starts, without competing with a_i
                    nc.sync.dma_start(out=item_sb[:], in_=item16[:])
                a_sl = apool.tile([P, GRP, IL], F16, tag="aslice")
                nc.sync.dma_start(
                    out=a_sl[:],
                    in_=a_i[ug * GRP * P:(ug + 1) * GRP * P, :].rearrange(
                        "(g p) i -> p g i", p=P),
                )
                for g in range(GRP):
                    uch = ug * GRP + g
                    for iw in range(NIW):
                        nc.tensor.matmul(
                            ps_iw[iw][:],
                            lhsT=user_sb[:, uch * P:(uch + 1) * P],
                            rhs=a_sl[:, g, iw * ILW:(iw + 1) * ILW],
                            start=(uch == 0), stop=(uch == NUC - 1),
                        )
            i_nT_loc = wpool.tile([D, IL], F32, tag="inTloc")
            for iw in range(NIW):
                nc.vector.tensor_copy(
                    out=i_nT_loc[:, iw * ILW:(iw + 1) * ILW], in_=ps_iw[iw][:])
            # W-apply then cast fp16 for the collective
            cc16 = wpool.tile([D, IL], F16, tag="cc16")
            for iw in range(NIW):
                ps = ps_seg.tile([D, ILW], F32, tag=f"segps{iw}")
                nc.tensor.matmul(
                    ps[:], lhsT=w_sb[:],
                    rhs=i_nT_loc[:, iw * ILW:(iw + 1) * ILW],
                    start=True, stop=True,
                )
                nc.vector.tensor_copy(
                    out=cc16[:, iw * ILW:(iw + 1) * ILW], in_=ps[:])
            nc.sync.dma_start(out=cc_in[:], in_=cc16[:])
            nc.gpsimd.collective_compute(
                "AllGather", mybir.AluOpType.bypass,
                ins=[cc_in[:].opt()],
                outs=[cc_out[:].opt()],
                replica_groups=[list(range(cores))],
            )

            # ---------- u-side segsum ----------
            ps_uw = [ps_seg.tile([D, UHW], F32, tag=f"segps{uw}",
                                 name=f"psuw{uw}")
                     for uw in range(NUH)]
            for ig in range(NIC // GRP):
                a_sl = apool.tile([P, GRP, UL], F16, tag="aslice")
                nc.sync.dma_start(
                    out=a_sl[:],
                    in_=a_ut[ig * GRP * P:(ig + 1) * GRP * P, :].rearrange(
                        "(g p) u -> p g u", p=P),
                )
                for g in range(GRP):
                    ich = ig * GRP + g
                    for uw in range(NUH):
                        nc.tensor.matmul(
                            ps_uw[uw][:],
                            lhsT=item_sb[:, ich * P:(ich + 1) * P],
                            rhs=a_sl[:, g, uw * UHW:(uw + 1) * UHW],
                            start=(ich == 0), stop=(ich == NIC - 1),
                        )
            u_rawT = wpool.tile([D, UL], F32, tag="urawT")
            for uw in range(NUH):
                nc.vector.tensor_copy(
                    out=u_rawT[:, uw * UHW:(uw + 1) * UHW], in_=ps_uw[uw][:])
            u_fin = wpool.tile([D, UL], F16, tag="ufin")
            for uw in range(NUH):
                ps = ps_seg.tile([D, UHW], F32, tag=f"segps{uw}")
                nc.tensor.matmul(
                    ps[:], lhsT=w_sb[:],
                    rhs=u_rawT[:, uw * UHW:(uw + 1) * UHW],
                    start=True, stop=True,
                )
                nc.vector.tensor_copy(
                    out=u_fin[:, uw * UHW:(uw + 1) * UHW], in_=ps[:])

            # ---------- itemT from item_sb via PE transposes ----------
            for c in range(NIC):
                ps_t = ps_seg.tile([P, P], F16, tag="segps2",
                                   name=f"itp{c}")
                nc.tensor.transpose(out=ps_t[:],
                                    in_=item_sb[:, c * P:(c + 1) * P],
                                    identity=ident16[:])
                nc.vector.tensor_copy(out=itemT_sb[:, c * P:(c + 1) * P],
                                      in_=ps_t[:])

            # ---------- W2 and ekwo (+ones) ----------
            ps_w2 = ps_seg.tile([D, D], F32, tag="segps0")
            nc.tensor.matmul(ps_w2[:], lhsT=wt_sb[:], rhs=w_sb[:],
                             start=True, stop=True)
            w2_16 = cpool.tile([D, D], F16)
            nc.vector.tensor_copy(out=w2_16[:], in_=ps_w2[:])
            ekwo = bigp.tile([P, NIC * (D + 1)], F16)
            nc.gpsimd.memset(ekwo[:], 1.0)
            for c in range(NIC):
                ps = ps_seg.tile([P, D], F32, tag="segps0")
                nc.tensor.matmul(
                    ps[:], lhsT=itemT_sb[:, c * P:(c + 1) * P], rhs=w2_16[:],
                    start=True, stop=True)
                nc.scalar.copy(
                    out=ekwo[:, c * (D + 1):c * (D + 1) + D], in_=ps[:])

            # ---------- gather i_neighT (fp16) -> upcast fp32 ----------
            i16 = bigp.tile([D, ni], F16)
            for cblk in range(cores):
                nc.sync.dma_start(
                    out=i16[:, cblk * IL:(cblk + 1) * IL],
                    in_=cc_out[cblk * D:(cblk + 1) * D, :],
                )

            # ---------- pass1 (max) + pass2 ----------
            # pass1 window 0 runs standalone (nothing to hide it under);
            # pass1 window 1 is interleaved 1:1 into pass2-w0's chunk loop.
            # pass2 issues each chunk's attV matmuls two chunks late so the
            # exp never sits on the tensor engine's critical path.
            negms = [negmp.tile([1, UHW], F16, tag="negm", name=f"negm{uw}")
                     for uw in range(NUH)]
            prts = {}

            def p1_mm(uw, k, ps):
                b, ib = k // NIB, k % NIB
                ub = uw * UBW + b
                if ib == 0:
                    prts[(uw, b)] = mtpool.tile([P, NIB], F32, tag="prt",
                                                name=f"prt_{uw}_{b}")
                nc.tensor.matmul(
                    ps[:],
                    lhsT=u_fin[:, ub * P:(ub + 1) * P],
                    rhs=i16[:, ib * 512:(ib + 1) * 512],
                    start=True, stop=True)
                nc.vector.reduce_max(out=prts[(uw, b)][:, ib:ib + 1],
                                     in_=ps[:], axis=AX)
                if ib == NIB - 1:
                    prt = prts.pop((uw, b))
                    mcol = mtpool.tile([P, 1], F32, tag="mcol")
                    nc.vector.reduce_max(out=mcol[:], in_=prt[:], axis=AX)
                    ps_tr = ps_seg.tile([1, P], F32, tag="segps0",
                                        name=f"ptr_{uw}_{b}")
                    nc.tensor.transpose(out=ps_tr[:], in_=mcol[:],
                                        identity=ident[:])
                    nc.vector.tensor_scalar_mul(
                        out=negms[uw][:, b * P:(b + 1) * P], in0=ps_tr[:],
                        scalar1=-1.0)

            # pass1 window 0 (exposed; deep psum rotation on the segps tags)
            for k in range(UBW * NIB):
                ps = ps_seg.tile([P, 512], F32, tag=f"segps{k % 3}",
                                 name=f"p1a_{k}")
                p1_mm(0, k, ps)

            for uw in range(NUH):
                negm = negms[uw]
                ops = [ps_out.tile([P, D + 1], F32, tag=f"ops{b}",
                                   name=f"ops_{uw}_{b}")
                       for b in range(UBW)]
                usl = slice(uw * UHW, (uw + 1) * UHW)
                ats = {}

                def attv(cc, uw=uw, ops=ops, ats=ats):
                    at_c = ats.pop(cc)
                    for b in range(UBW):
                        nc.tensor.matmul(
                            ops[b][:],
                            lhsT=at_c[:, b * P:(b + 1) * P],
                            rhs=ekwo[:, cc * (D + 1):(cc + 1) * (D + 1)],
                            start=(cc == 0), stop=(cc == NIC - 1))

                for c in range(NIC):
                    ps = ps_seg.tile([P, UHW], F32, tag=f"segps{c % 3}",
                                     name=f"lg_{uw}_{c}")
                    csl = slice(c * P, (c + 1) * P)
                    nc.tensor.matmul(
                        ps[:], lhsT=i16[:, csl], rhs=u_fin[:, usl],
                        start=True, stop=False)
                    nc.tensor.matmul(
                        ps[:], lhsT=ones1[:], rhs=negm[:],
                        start=False, stop=True)
                    at = atpool.tile([P, UHW], F16, tag="at")
                    ats[c] = at
                    nc.scalar.activation(at[:], ps[:], AF.Exp)
                    if uw == 0 and NUH > 1:
                        ps1 = ps1pool.tile([P, 512], F32, tag="p1ps")
                        p1_mm(1, c, ps1)
                    if c >= 2:
                        attv(c - 2)
                attv(NIC - 2)
                attv(NIC - 1)

                # ---- finalize: divide by Z, store
                for b in range(UBW):
                    ub = uw * UBW + b
                    src = ops[b][:]
                    rec = mtpool.tile([P, 1], F32, tag="rec")
                    nc.vector.reciprocal(out=rec[:], in_=src[:, D:D + 1])
                    o_sb = mtpool.tile([P, D], F32, tag="osb")
                    nc.vector.tensor_scalar_mul(
                        out=o_sb[:], in0=src[:, 0:D], scalar1=rec[:])
                    nc.sync.dma_start(
                        out=out_sl[ub * P:(ub + 1) * P, :], in_=o_sb[:])

    nc.compile()
    return nc


def _prep_inputs(user_emb, item_emb, W, adj_val, adj_row, adj_col, cores):
    nu, d = user_emb.shape
    ni = item_emb.shape[0]
    UL, IL = nu // cores, ni // cores
    A = np.zeros((nu, ni), np.float32)
    np.add.at(A, (adj_row, adj_col), adj_val)
    A16 = A.astype(np.float16)
    AT16 = np.ascontiguousarray(A16.T)
    item16 = item_emb.astype(np.float16)
    itemT16 = np.ascontiguousarray(item16.T)
    user16 = user_emb.astype(np.float16)
    # [n, D] -> [P, nchunks*D] partition-major chunk layout (device lhsT form)
    item16_dev = np.ascontiguousarray(
        item16.reshape(ni // 128, 128, d).transpose(1, 0, 2).reshape(128, -1))
    user16_dev = np.ascontiguousarray(
        user16.reshape(nu // 128, 128, d).transpose(1, 0, 2).reshape(128, -1))
    W = np.ascontiguousarray(W, np.float32)
    WT = np.ascontiguousarray(W.T)
    in_maps = []
    for m in range(cores):
        in_maps.append({
            "a_ut": np.ascontiguousarray(AT16[:, m * UL:(m + 1) * UL]),
            "a_i": np.ascontiguousarray(A16[:, m * IL:(m + 1) * IL]),
            "item16": item16_dev,
            "itemT16": itemT16,
            "user16": user16_dev,
            "w": W,
            "wt": WT,
        })
    return in_maps


_NC_CACHE = {}


def _ensure_ntff_hook():
    """Register the axon NTFF profiling hook (image's antenv lacks it)."""
    import types
    if 'antenv.axon_hooks' not in sys.modules:
        mod = types.ModuleType('antenv.axon_hooks')
        _h = {'v': None}
        mod.set_axon_ntff_profile_hook = lambda h: _h.__setitem__('v', h)
        mod.get_axon_ntff_profile_hook = lambda: _h['v']
        sys.modules['antenv.axon_hooks'] = mod
        try:
            import antenv
            antenv.axon_hooks = mod
        except ImportError:
            pass
    mod = sys.modules['antenv.axon_hooks']
    if mod.get_axon_ntff_profile_hook() is None:
        try:
            sys.path.insert(0, '/root/.axon_site/trn_agent_boot')
            import trn_boot
            hook = trn_boot._ntff_profile_via_ctypes('/opt/axon/libaxon_pjrt.so')
            if hook is not None:
                mod.set_axon_ntff_profile_hook(hook)
        except Exception:
            pass
    bass_utils.upload_artifacts = lambda d: d


def kernel(user_emb, item_emb, W, adj_val, adj_row, adj_col,
           cores=8, _trace=False):
    user_emb = np.asarray(user_emb, np.float32)
    item_emb = np.asarray(item_emb, np.float32)
    W = np.asarray(W, np.float32)
    adj_val = np.asarray(adj_val, np.float32)
    adj_row = np.asarray(adj_row, np.int32)
    adj_col = np.asarray(adj_col, np.int32)

    nu, ni = user_emb.shape[0], item_emb.shape[0]
    key = (nu, ni, cores)
    if key not in _NC_CACHE:
        _NC_CACHE[key] = build_nc(nu, ni, cores)
    nc = _NC_CACHE[key]

    in_maps = _prep_inputs(user_emb, item_emb, W, adj_val, adj_row,
                           adj_col, cores)
    if _trace:
        _ensure_ntff_hook()
        try:
            res = bass_utils.run_bass_kernel_spmd(
                nc, in_maps, core_ids=list(range(cores)), trace=True)
        except Exception as e:
            print(f"trace run failed ({e!r}); retrying without trace",
                  flush=True)
            res = bass_utils.run_bass_kernel_spmd(
                nc, in_maps, core_ids=list(range(cores)), trace=False)
    else:
        res = bass_utils.run_bass_kernel_spmd(
            nc, in_maps, core_ids=list(range(cores)), trace=False)
    UL = nu // cores
    out = np.empty((nu, 128), np.float32)
    for m in range(cores):
        out[m * UL:(m + 1) * UL] = res.results[m]["out_sl"]
    if _trace:
        kernel._last_res = res
    return out



# revision 8
# speedup vs baseline: 1.4356x; 1.4356x over previous
"""Trainium2 Bass kernel for nn_AttenConv (sparse_attention).

Math (see reference):
  e_j = user_emb @ W ; e_k = item_emb @ W
  u_neigh = segsum_row(adj, e_k) ; i_neigh = segsum_col(adj, e_j)
  out = softmax(u_neigh @ i_neigh.T, axis=1) @ e_k @ W

Kernel formulation:
  A (dense [U,I], fp16) built on host from COO.  W commutes with segsum:
    u_neigh = (A @ item_emb) @ W ; i_neigh = (A.T @ user_emb) @ W
  ekW = item_emb @ (W @ W).
  Sharding: users row-sharded (u-side), items col-sharded (i-side segsum)
  with AllGather of i_neighT; attention fully user-sharded.

v4 schedule (vs v3):
  - eigen bias-kill: logits contraction goes through M = W W^T.  Factor
    M = B B^T with B [128,127] (drop smallest eigenvalue; max logit err
    ~8e-3 vs top-2 gaps ~51).  Apply B^T (padded with a zero 128th
    column) instead of W^T after both segsums; partition row 127 of
    u_fin then carries -rowmax and row 127 of i16 carries 1.0, so the
    pass-2 logits matmul computes (logits - max) directly.  This deletes
    the 128 rank-1 bias matmuls (~600ns each on HW).
  - a_ut is split into two contiguous column-halves on host; u-side
    segsum completes per 512-user window, so pass1 of window 0 runs
    under the window-1 adjacency stream.
  - itemT transposes + W2 + ekwo moved into the (DMA-bound) a_ut0
    stream window where the PE has slack.
"""
import sys
sys.path.insert(0, '/opt/trn_rl_repo')
import numpy as np

import concourse.bass as bass
import concourse.mybir as mybir
import concourse.tile as tile
from concourse import bacc, bass_utils
from concourse.masks import make_identity

F32 = mybir.dt.float32
F16 = mybir.dt.float16
AX = mybir.AxisListType.X
AF = mybir.ActivationFunctionType

D = 128
P = 128


def build_nc(nu, ni, cores):
    UL = nu // cores          # users per core
    IL = ni // cores          # items per core (i-side segsum shard)
    UHW = min(512, UL)        # user window for pass1/2
    NUH = UL // UHW           # user windows per core
    UBW = UHW // P            # 128-user blocks per window
    NIC = ni // P             # 128-item chunks
    NUC = nu // P             # 128-user chunks (i-side segsum contraction)
    ILW = min(512, IL)
    NIW = IL // ILW
    NIB = ni // 512           # 512-item blocks (pass-1 rhs)

    nc = bacc.Bacc("TRN2", debug=False, num_devices=cores)

    a_uts = [nc.dram_tensor(f"a_ut{w}", [ni, UHW], F16, kind="ExternalInput")
             for w in range(NUH)]
    a_i = nc.dram_tensor("a_i", [nu, IL], F16, kind="ExternalInput")
    item16 = nc.dram_tensor("item16", [P, ni], F16, kind="ExternalInput")
    user16 = nc.dram_tensor("user16", [P, nu], F16, kind="ExternalInput")
    w_in = nc.dram_tensor("w", [D, D], F32, kind="ExternalInput")
    wt_in = nc.dram_tensor("wt", [D, D], F32, kind="ExternalInput")
    bpad_in = nc.dram_tensor("bpad", [D, D], F32, kind="ExternalInput")
    out_sl = nc.dram_tensor("out_sl", [UL, D], F32, kind="ExternalOutput")

    cc_in = nc.dram_tensor("cc_in", [D, IL], F16)
    cc_out = nc.dram_tensor("cc_out", [cores * D, IL], F16,
                            addr_space="Shared")

    with tile.TileContext(nc) as tc:
        with (
            tc.tile_pool(name="const", bufs=1) as cpool,
            tc.tile_pool(name="big", bufs=1) as bigp,
            tc.tile_pool(name="astream", bufs=5) as apool,
            tc.tile_pool(name="work", bufs=1) as wpool,
            tc.tile_pool(name="mt", bufs=2) as mtpool,
            tc.tile_pool(name="at", bufs=4) as atpool,
            tc.tile_pool(name="ps1", bufs=1, space="PSUM") as ps1pool,
            tc.tile_pool(name="ps_seg", bufs=1, space="PSUM") as ps_seg,
            tc.tile_pool(name="ps_out", bufs=1, space="PSUM") as ps_out,
        ):
            # ---------- constants + user emb (needed by i-side segsum) ----
            w_sb = cpool.tile([D, D], F32)
            wt_sb = cpool.tile([D, D], F32)
            bpad_sb = cpool.tile([D, D], F32)
            nc.sync.dma_start(out=w_sb[:], in_=w_in[:])
            nc.sync.dma_start(out=wt_sb[:], in_=wt_in[:])
            nc.sync.dma_start(out=bpad_sb[:], in_=bpad_in[:])
            ident = cpool.tile([P, P], F32)
            make_identity(nc, ident[:])
            ident16 = cpool.tile([P, P], F16)
            make_identity(nc, ident16[:])
            user_sb = bigp.tile([P, nu], F16)
            nc.sync.dma_start(out=user_sb[:], in_=user16[:])

            # ---------- i-side segsum (feeds AllGather; a_i has DMA
            # priority: no other bulk DMA is issued before it) ----------
            GRP = 4 if NUC % 4 == 0 else 1
            ps_iw = [ps_seg.tile([D, ILW], F32, tag=f"segps{iw}",
                                 name=f"psiw{iw}")
                     for iw in range(NIW)]
            item_sb = bigp.tile([P, ni], F16)
            itemT_sb = bigp.tile([P, ni], F16)
            for ug in range(NUC // GRP):
                if ug == NUC // GRP - 6:
                    # issue late in the a_i stream: ready right when the
                    # u-side segsum starts, without competing with a_i
                    nc.sync.dma_start(out=item_sb[:], in_=item16[:])
                a_sl = apool.tile([P, GRP, IL], F16, tag="aslice")
                nc.sync.dma_start(
                    out=a_sl[:],
                    in_=a_i[ug * GRP * P:(ug + 1) * GRP * P, :].rearrange(
                        "(g p) i -> p g i", p=P),
                )
                for g in range(GRP):
                    uch = ug * GRP + g
                    for iw in range(NIW):
                        nc.tensor.matmul(
                            ps_iw[iw][:],
                            lhsT=user_sb[:, uch * P:(uch + 1) * P],
                            rhs=a_sl[:, g, iw * ILW:(iw + 1) * ILW],
                            start=(uch == 0), stop=(uch == NUC - 1),
                        )
            i_nT_loc = wpool.tile([D, IL], F32, tag="inTloc")
            for iw in range(NIW):
                nc.vector.tensor_copy(
                    out=i_nT_loc[:, iw * ILW:(iw + 1) * ILW], in_=ps_iw[iw][:])
            # B-apply then cast fp16 for the collective (row 127 -> 0)
            cc16 = wpool.tile([D, IL], F16, tag="cc16")
            for iw in range(NIW):
                ps = ps_seg.tile([D, ILW], F32, tag=f"segps{iw}")
                nc.tensor.matmul(
                    ps[:], lhsT=bpad_sb[:],
                    rhs=i_nT_loc[:, iw * ILW:(iw + 1) * ILW],
                    start=True, stop=True,
                )
                nc.vector.tensor_copy(
                    out=cc16[:, iw * ILW:(iw + 1) * ILW], in_=ps[:])
            # softmax-bias carrier: row 0 of i_neighT is all-ones (the
            # B-apply left it exactly zero); the AllGather replicates it.
            nc.vector.tensor_scalar_add(out=cc16[0:1, :], in0=cc16[0:1, :],
                                        scalar1=1.0)
            nc.sync.dma_start(out=cc_in[:], in_=cc16[:])
            nc.gpsimd.collective_compute(
                "AllGather", mybir.AluOpType.bypass,
                ins=[cc_in[:].opt()],
                outs=[cc_out[:].opt()],
                replica_groups=[list(range(cores))],
            )

            # ---------- gather i_neighT (fp16); row 127 -> ones ----------
            i16 = bigp.tile([D, ni], F16)
            for cblk in range(cores):
                nc.sync.dma_start(
                    out=i16[:, cblk * IL:(cblk + 1) * IL],
                    in_=cc_out[cblk * D:(cblk + 1) * D, :],
                )

            # ---------- u-side segsum, per window; window0 B-applied
            # early so pass1(w0) runs under the window-1 stream ----------
            u_fin = wpool.tile([D, UL], F16, tag="ufin")
            u_rawT = wpool.tile([D, UHW], F32, tag="urawT")
            prts = {}

            def p1_mm(uw, k, ps):
                """pass1: one [128u x 512i] logits tile + row-max chain.
                Writes -max into u_fin row 127 at this block's columns."""
                b, ib = k // NIB, k % NIB
                ub = uw * UBW + b
                if ib == 0:
                    prts[(uw, b)] = mtpool.tile([P, NIB], F32, tag="prt",
                                                name=f"prt_{uw}_{b}")
                nc.tensor.matmul(
                    ps[:],
                    lhsT=u_fin[:, ub * P:(ub + 1) * P],
                    rhs=i16[:, ib * 512:(ib + 1) * 512],
                    start=True, stop=True)
                nc.vector.reduce_max(out=prts[(uw, b)][:, ib:ib + 1],
                                     in_=ps[:], axis=AX)
                if ib == NIB - 1:
                    prt = prts.pop((uw, b))
                    mcol = mtpool.tile([P, 1], F32, tag="mcol")
                    nc.vector.reduce_max(out=mcol[:], in_=prt[:], axis=AX)
                    ps_tr = ps_seg.tile([1, P], F32, tag="segps0",
                                        name=f"ptr_{uw}_{b}")
                    nc.tensor.transpose(out=ps_tr[:], in_=mcol[:],
                                        identity=ident[:])
                    nc.vector.tensor_scalar_mul(
                        out=u_fin[0:1, ub * P:(ub + 1) * P],
                        in0=ps_tr[:], scalar1=-1.0)

            w2_16 = cpool.tile([D, D], F16)
            ekwo = bigp.tile([P, NIC * (D + 1)], F16)
            nc.gpsimd.memset(ekwo[:], 1.0)

            NG = NIC // GRP  # dma groups per window stream
            for w in range(NUH):
                ps_uw = ps_seg.tile([D, UHW], F32, tag="segps1",
                                    name=f"psuw{w}")
                for ig in range(NG):
                    a_sl = apool.tile([P, GRP, UHW], F16, tag="aslice")
                    nc.sync.dma_start(
                        out=a_sl[:],
                        in_=a_uts[w][ig * GRP * P:(ig + 1) * GRP * P, :]
                        .rearrange("(g p) u -> p g u", p=P),
                    )
                    for g in range(GRP):
                        ich = ig * GRP + g
                        nc.tensor.matmul(
                            ps_uw[:],
                            lhsT=item_sb[:, ich * P:(ich + 1) * P],
                            rhs=a_sl[:, g, :],
                            start=(ich == 0), stop=(ich == NIC - 1),
                        )
                    if w == 0:
                        # fill PE slack under the DMA-bound w0 stream:
                        # itemT transposes, then W2, then ekwo (lagging
                        # one group behind the transposes)
                        for g in range(GRP):
                            c = ig * GRP + g
                            ps_t = ps_seg.tile([P, P], F16, tag="segps2",
                                               name=f"itp{c}")
                            nc.tensor.transpose(
                                out=ps_t[:],
                                in_=item_sb[:, c * P:(c + 1) * P],
                                identity=ident16[:])
                            nc.vector.tensor_copy(
                                out=itemT_sb[:, c * P:(c + 1) * P],
                                in_=ps_t[:])
                        if ig == 0:
                            ps_w2 = ps_seg.tile([D, D], F32, tag="segps0")
                            nc.tensor.matmul(ps_w2[:], lhsT=wt_sb[:],
                                             rhs=w_sb[:],
                                             start=True, stop=True)
                            nc.vector.tensor_copy(out=w2_16[:], in_=ps_w2[:])
                        else:
                            for g in range(GRP):
                                c = (ig - 1) * GRP + g
                                ps = ps_seg.tile([P, D], F32, tag="segps0",
                                                 name=f"ekw{c}")
                                nc.tensor.matmul(
                                    ps[:],
                                    lhsT=itemT_sb[:, c * P:(c + 1) * P],
                                    rhs=w2_16[:], start=True, stop=True)
                                nc.scalar.copy(
                                    out=ekwo[:, c * (D + 1):c * (D + 1) + D],
                                    in_=ps[:])
                    else:
                        # interleave pass1(w0) under the w1 stream
                        for g in range(GRP):
                            k = ig * GRP + g
                            pool = ps1pool if k % 2 == 0 else ps_seg
                            ps = pool.tile(
                                [P, 512], F32,
                                tag="p1ps" if k % 2 == 0 else "segps2",
                                name=f"p1a_{k}")
                            p1_mm(0, k, ps)
                # last ekwo group (lagged by one)
                if w == 0:
                    for g in range(GRP):
                        c = (NG - 1) * GRP + g
                        ps = ps_seg.tile([P, D], F32, tag="segps0",
                                         name=f"ekw{c}")
                        nc.tensor.matmul(
                            ps[:], lhsT=itemT_sb[:, c * P:(c + 1) * P],
                            rhs=w2_16[:], start=True, stop=True)
                        nc.scalar.copy(
                            out=ekwo[:, c * (D + 1):c * (D + 1) + D],
                            in_=ps[:])
                # B-apply this window -> u_fin (row 127 = 0)
                nc.vector.tensor_copy(out=u_rawT[:], in_=ps_uw[:])
                ps = ps_seg.tile([D, UHW], F32, tag="segps1",
                                 name=f"bap{w}")
                nc.tensor.matmul(ps[:], lhsT=bpad_sb[:], rhs=u_rawT[:],
                                 start=True, stop=True)
                nc.vector.tensor_copy(
                    out=u_fin[:, w * UHW:(w + 1) * UHW], in_=ps[:])

            # ---------- pass2 (+ pass1 of w1 interleaved into w0) ----------
            for uw in range(NUH):
                ops = [ps_out.tile([P, D + 1], F32, tag=f"ops{b}",
                                   name=f"ops_{uw}_{b}")
                       for b in range(UBW)]
                usl = slice(uw * UHW, (uw + 1) * UHW)
                ats = {}

                def attv(cc, uw=uw, ops=ops, ats=ats):
                    at_c = ats.pop(cc)
                    for b in range(UBW):
                        nc.tensor.matmul(
                            ops[b][:],
                            lhsT=at_c[:, b * P:(b + 1) * P],
                            rhs=ekwo[:, cc * (D + 1):(cc + 1) * (D + 1)],
                            start=(cc == 0), stop=(cc == NIC - 1))

                for c in range(NIC):
                    ps = ps_seg.tile([P, UHW], F32, tag=f"segps{c % 3}",
                                     name=f"lg_{uw}_{c}")
                    csl = slice(c * P, (c + 1) * P)
                    nc.tensor.matmul(
                        ps[:], lhsT=i16[:, csl], rhs=u_fin[:, usl],
                        start=True, stop=True)
                    at = atpool.tile([P, UHW], F16, tag="at")
                    ats[c] = at
                    nc.scalar.activation(at[:], ps[:], AF.Exp)
                    if uw == 0 and NUH > 1:
                        ps1 = ps1pool.tile([P, 512], F32, tag="p1ps")
                        p1_mm(1, c, ps1)
                    if c >= 2:
                        attv(c - 2)
                attv(NIC - 2)
                attv(NIC - 1)

                # ---- finalize: divide by Z, store
                for b in range(UBW):
                    ub = uw * UBW + b
                    src = ops[b][:]
                    rec = mtpool.tile([P, 1], F32, tag="rec")
                    nc.vector.reciprocal(out=rec[:], in_=src[:, D:D + 1])
                    o_sb = mtpool.tile([P, D], F32, tag="osb")
                    nc.vector.tensor_scalar_mul(
                        out=o_sb[:], in0=src[:, 0:D], scalar1=rec[:])
                    nc.sync.dma_start(
                        out=out_sl[ub * P:(ub + 1) * P, :], in_=o_sb[:])

    nc.compile()
    return nc


def _prep_inputs(user_emb, item_emb, W, adj_val, adj_row, adj_col, cores):
    nu, d = user_emb.shape
    ni = item_emb.shape[0]
    UL, IL = nu // cores, ni // cores
    UHW = min(512, UL)
    NUH = UL // UHW
    A = np.zeros((nu, ni), np.float32)
    np.add.at(A, (adj_row, adj_col), adj_val)
    A16 = A.astype(np.float16)
    AT16 = np.ascontiguousarray(A16.T)
    item16 = item_emb.astype(np.float16)
    user16 = user_emb.astype(np.float16)
    # [n, D] -> [P, nchunks*D] partition-major chunk layout (device lhsT form)
    item16_dev = np.ascontiguousarray(
        item16.reshape(ni // 128, 128, d).transpose(1, 0, 2).reshape(128, -1))
    user16_dev = np.ascontiguousarray(
        user16.reshape(nu // 128, 128, d).transpose(1, 0, 2).reshape(128, -1))
    W = np.ascontiguousarray(W, np.float32)
    WT = np.ascontiguousarray(W.T)
    # eigen factor of M = W W^T with the smallest eigenvalue dropped;
    # zero 128th column frees partition row 127 for the softmax bias.
    M = (W.astype(np.float64) @ W.astype(np.float64).T)
    lam, Q = np.linalg.eigh(M)
    lam = np.maximum(lam, 0.0)
    B = Q[:, 1:] * np.sqrt(lam[1:])
    bpad = np.zeros((d, d), np.float32)
    bpad[:, 1:] = B.astype(np.float32)   # column 0 zero: frees row 0
    bpad = np.ascontiguousarray(bpad)
    in_maps = []
    for m in range(cores):
        im = {
            "a_i": np.ascontiguousarray(A16[:, m * IL:(m + 1) * IL]),
            "item16": item16_dev,
            "user16": user16_dev,
            "w": W,
            "wt": WT,
            "bpad": bpad,
        }
        for w in range(NUH):
            c0 = m * UL + w * UHW
            im[f"a_ut{w}"] = np.ascontiguousarray(AT16[:, c0:c0 + UHW])
        in_maps.append(im)
    return in_maps


_NC_CACHE = {}


def _ensure_ntff_hook():
    """Register the axon NTFF profiling hook (image's antenv lacks it)."""
    import types
    if 'antenv.axon_hooks' not in sys.modules:
        mod = types.ModuleType('antenv.axon_hooks')
        _h = {'v': None}
        mod.set_axon_ntff_profile_hook = lambda h: _h.__setitem__('v', h)
        mod.get_axon_ntff_profile_hook = lambda: _h['v']
        sys.modules['antenv.axon_hooks'] = mod
        try:
            import antenv
            antenv.axon_hooks = mod
        except ImportError:
            pass
    mod = sys.modules['antenv.axon_hooks']
    if mod.get_axon_ntff_profile_hook() is None:
        try:
            sys.path.insert(0, '/root/.axon_site/trn_agent_boot')
            import trn_boot
            hook = trn_boot._ntff_profile_via_ctypes('/opt/axon/libaxon_pjrt.so')
            if hook is not None:
                mod.set_axon_ntff_profile_hook(hook)
        except Exception:
            pass
    bass_utils.upload_artifacts = lambda d: d


def kernel(user_emb, item_emb, W, adj_val, adj_row, adj_col,
           cores=8, _trace=False):
    user_emb = np.asarray(user_emb, np.float32)
    item_emb = np.asarray(item_emb, np.float32)
    W = np.asarray(W, np.float32)
    adj_val = np.asarray(adj_val, np.float32)
    adj_row = np.asarray(adj_row, np.int32)
    adj_col = np.asarray(adj_col, np.int32)

    nu, ni = user_emb.shape[0], item_emb.shape[0]
    key = (nu, ni, cores)
    if key not in _NC_CACHE:
        _NC_CACHE[key] = build_nc(nu, ni, cores)
    nc = _NC_CACHE[key]

    in_maps = _prep_inputs(user_emb, item_emb, W, adj_val, adj_row,
                           adj_col, cores)
    if _trace:
        _ensure_ntff_hook()
        try:
            res = bass_utils.run_bass_kernel_spmd(
                nc, in_maps, core_ids=list(range(cores)), trace=True)
        except Exception as e:
            print(f"trace run failed ({e!r}); retrying without trace",
                  flush=True)
            res = bass_utils.run_bass_kernel_spmd(
                nc, in_maps, core_ids=list(range(cores)), trace=False)
    else:
        res = bass_utils.run_bass_kernel_spmd(
            nc, in_maps, core_ids=list(range(cores)), trace=False)
    UL = nu // cores
    out = np.empty((nu, 128), np.float32)
    for m in range(cores):
        out[m * UL:(m + 1) * UL] = res.results[m]["out_sl"]
    if _trace:
        kernel._last_res = res
    return out
